# revision 33
# baseline (speedup 1.0000x reference)
"""Trainium2 Bass kernel for nn_AddingToQ (GNN message passing + sinkhorn).

Self-contained: takes FULL unsharded inputs, shards 256 graph pairs across
8 NeuronCores (32 pairs / 1920 nodes / 4320 real edges per core), runs an
all-SBUF matmul-formulated GNN, gathers per-core scores to the full [256]
output.

v2 restructuring (validated in numpy against the jax reference):
  * all-f32r propagation (fp32 bits, 1-pass PE rate at free>=256) vs the
    old fp32 node MLPs (4 cycles/row).
  * node tiles = 2 pairs compacted to 120 rows; per-block edge tiles (2 full
    128-tiles + packed remainder) -> every full edge tile gathers from
    exactly one node tile (48 incidences/direction vs 62).
  * msg-MLP edge constant c1 folded into row 120 of the U/V tiles: the
    gather one-hots carry an extra ones-row, so c1 costs zero instructions.
  * sinkhorn runs in the linear domain (row-max exp once, then 10 rounds of
    row/col divide) -> no exp/ln alternation, 2 act-table loads total.
  * final stage stays true-fp32 (precision: the 1/temp=10x logit scale
    amplifies any rounding into the transport plan).
"""
import numpy as np

# problem constants
B, NQ, NC = 256, 15, 30
NPG = 2 * NC
N = B * NPG
EPP = 135                 # real (mask=1) edges per pair
E_REAL = B * EPP
D, H, T = 128, 256, 64
N_PROP, SK_ITERS, SK_TEMP = 5, 10, 0.1
NCORES = 8
BP = B // NCORES          # 32 pairs per core
NL = BP * NPG             # 1920 nodes per core
EL = BP * EPP             # 4320 edges per core
NBLK = BP // 2            # 16 blocks (2 pairs = 120 nodes, 270 edges)
NFT = 32                  # full edge tiles (2 per block)
NRT = 2                   # remainder tiles (8 blocks x 14 edges = 112)
ET = NFT + NRT
NGU = NFT + 16            # gather incidences per direction
GS_COLS = NFT * 120 + 4 * 480   # scatter one-hot columns

_CACHE = {}


def _host_prep(inputs):
    f32 = np.float32
    msg_w1 = np.asarray(inputs['msg_w1'], f32)
    W1a, W1b, W1c = msg_w1[0:128], msg_w1[128:256], msg_w1[256:384]
    upd_w1 = np.asarray(inputs['upd_w1'], f32)
    A1, A2 = upd_w1[0:128], upd_w1[128:256]
    msg_w2 = np.asarray(inputs['msg_w2'], f32)
    M1 = (msg_w2 @ A1).astype(f32)
    b2A1 = (np.asarray(inputs['msg_b2'], f32) @ A1).astype(f32)
    upd_b1 = np.asarray(inputs['upd_b1'], f32)
    upd_w2 = np.asarray(inputs['upd_w2'], f32)
    upd_b2 = np.asarray(inputs['upd_b2'], f32)

    nf = np.asarray(inputs['node_features'], f32)
    h0 = nf * np.asarray(inputs['enc_node_w'], f32)[0][None, :] \
        + np.asarray(inputs['enc_node_b'], f32)[None, :]
    ef = np.asarray(inputs['edge_features'], f32)
    e_enc = ef * np.asarray(inputs['enc_edge_w'], f32)[0][None, :] \
        + np.asarray(inputs['enc_edge_b'], f32)[None, :]
    C_all = (e_enc @ W1c + np.asarray(inputs['msg_b1'], f32)[None, :]).astype(f32)
    assert bool(np.all(C_all[:E_REAL] == C_all[0])), "edge encodings not uniform"
    c1h = 0.5 * C_all[0]

    from_idx = np.asarray(inputs['from_idx']).astype(np.int64)
    to_idx = np.asarray(inputs['to_idx']).astype(np.int64)
    mask = np.asarray(inputs['mask_from_idx'], f32)
    assert np.all(mask[:E_REAL] == 1.0) and np.all(mask[E_REAL:] == 0.0)
    pair_of_edge = np.arange(E_REAL) // EPP
    assert np.all(from_idx[:E_REAL] // NPG == pair_of_edge)
    assert np.all(to_idx[:E_REAL] // NPG == pair_of_edge)

    # weights in exact SBUF layouts (same for all cores)
    w1ab = np.concatenate([W1a, W1b], axis=1)                     # [128, 512]
    m1 = np.concatenate([M1[0:128], M1[128:256]], axis=1)         # [128, 512]
    wu2 = np.concatenate([upd_w2[0:128], upd_w2[128:256]], axis=1)  # [128,256]
    updb1 = np.stack([upd_b1[0:128], upd_b1[128:256]], axis=1)    # [128, 2]
    c1pad = np.zeros((8, 16 * 512), f32)    # UV rows 120..127 (row 0 = c1/2)
    for k in range(16):
        c1pad[0, 512*k:512*k+256] = c1h
        c1pad[0, 512*k+256:512*k+512] = c1h
    # sinkhorn column-sum-broadcast ones (with junk-col fix) and score ones
    onesbd = np.zeros((128, 128), f32)
    onesq = np.zeros((128, 4), f32)
    for j in range(4):
        # junk cols (s>=30) get the same pattern: block colsums are positive,
        # so junk rows stay finite across iterations
        for s in range(32):
            onesbd[32*j:32*j+30, 32*j+s] = 1.0
        onesq[32*j:32*j+30, j] = 1.0

    common = {
        'w1ab': np.ascontiguousarray(w1ab), 'm1': np.ascontiguousarray(m1),
        'a2': np.ascontiguousarray(A2), 'wu2': np.ascontiguousarray(wu2),
        'b2a1': np.ascontiguousarray(b2A1[None, :]),
        'ub2': np.ascontiguousarray(upd_b2[None, :]),
        'updb1': np.ascontiguousarray(updb1),
        'c1pad': c1pad,
        'ft1': np.ascontiguousarray(np.asarray(inputs['ft1_w'], f32)),
        'ft2': np.ascontiguousarray(np.asarray(inputs['ft2_w'], f32)),
        'ft1b': np.ascontiguousarray(np.asarray(inputs['ft1_b'], f32)[:, None]),
        'ft2b': np.ascontiguousarray(np.asarray(inputs['ft2_b'], f32)[:, None]),
        'onesbd': onesbd, 'onesq': onesq,
    }

    in_maps = []
    for c in range(NCORES):
        n0, e0 = c * NL, c * EL
        fl = from_idx[e0:e0 + EL] - n0
        tl = to_idx[e0:e0 + EL] - n0
        assert fl.min() >= 0 and fl.max() < NL and tl.min() >= 0 and tl.max() < NL

        gu = np.zeros((128, NGU * 128), f32)
        gv = np.zeros((128, NGU * 128), f32)
        gs = np.zeros((128, GS_COLS), f32)
        for t in range(NFT):
            b, i = t // 2, t % 2
            es = slice(270*b + 128*i, 270*b + 128*i + 128)
            flb, tlb = fl[es] - 120*b, tl[es] - 120*b
            cols = np.arange(128)
            gu[flb, t*128 + cols] = 1.0
            gv[tlb, t*128 + cols] = 1.0
            gu[120, t*128:(t+1)*128] = 1.0
            gv[120, t*128:(t+1)*128] = 1.0
            gs[cols, t*120 + tlb] = 1.0
        for rt in range(NRT):
            for kk in range(8):
                bb = 8*rt + kk
                idx = NFT + 8*rt + kk
                js = 14*kk + np.arange(14)
                es = 270*bb + 256 + np.arange(14)
                flb, tlb = fl[es] - 120*bb, tl[es] - 120*bb
                gu[flb, idx*128 + js] = 1.0
                gv[tlb, idx*128 + js] = 1.0
                gu[120, idx*128 + js] = 1.0
                gv[120, idx*128 + js] = 1.0
                gg = bb // 4             # target group
                gcol = NFT*120 + (2*rt + (gg % 2)) * 480
                gs[js, gcol + 120*(bb % 4) + tlb] = 1.0

        indeg = np.zeros(NL, f32)
        np.add.at(indeg, tl, 1.0)

        m = {
            'ht0': np.ascontiguousarray(h0[n0:n0 + NL].T),        # [128,1920]
            'gu': gu, 'gv': gv, 'gs': gs,
            'indeg': np.ascontiguousarray(indeg[None, :]),
        }
        m.update(common)
        in_maps.append(m)
    return in_maps


def _build():
    """Build + schedule the Bass/Tile program (identical for all cores)."""
    import concourse.bass as bass
    import concourse.tile as tile
    from concourse import bacc, mybir
    from concourse.masks import make_identity

    f32 = mybir.dt.float32
    f32r = mybir.dt.float32r
    AF = mybir.ActivationFunctionType
    ALU = mybir.AluOpType
    AX = mybir.AxisListType

    nc = bacc.Bacc("TRN2", target_bir_lowering=False, debug=False)

    dram = {}
    def din(name, shape, dt_=f32):
        dram[name] = nc.dram_tensor(name, list(shape), dt_,
                                    kind="ExternalInput").ap()
    din('ht0', (128, NL))
    din('gu', (128, NGU * 128), f32r)
    din('gv', (128, NGU * 128), f32r)
    din('gs', (128, GS_COLS), f32r)
    din('indeg', (1, NL), f32r)
    din('w1ab', (128, 512), f32r); din('m1', (128, 512), f32r)
    din('a2', (128, H), f32r); din('wu2', (128, H), f32r)
    din('b2a1', (1, H), f32r); din('ub2', (1, 128), f32r)
    din('updb1', (128, 2))
    din('c1pad', (8, 16 * 512), f32r)
    din('ft1', (128, T)); din('ft2', (T, T))
    din('ft1b', (T, 1)); din('ft2b', (T, 1))
    din('onesbd', (128, 128)); din('onesq', (128, 4))
    scores_out = nc.dram_tensor('scores', [4, 8], f32, kind="ExternalOutput").ap()
    import os
    DBG = bool(os.environ.get('KERNEL_DEBUG'))
    if DBG:
        dbg_h = nc.dram_tensor('dbg_h', [128, NL], f32, kind="ExternalOutput").ap()
        dbg_al0 = nc.dram_tensor('dbg_al0', [128, 240], f32, kind="ExternalOutput").ap()
        dbg_al = nc.dram_tensor('dbg_al', [128, 240], f32, kind="ExternalOutput").ap()
        dbg_uv = nc.dram_tensor('dbg_uv', [128, 8192], mybir.dt.float32r, kind="ExternalOutput").ap()
        dbg_agg = nc.dram_tensor('dbg_agg', [128, 960], mybir.dt.float32r, kind="ExternalOutput").ap()
        dbg_rs = nc.dram_tensor('dbg_rs', [128, 8], f32, kind="ExternalOutput").ap()
        dbg_rr = nc.dram_tensor('dbg_rr', [128, 8], f32, kind="ExternalOutput").ap()
        dbg_alr = nc.dram_tensor('dbg_alr', [128, 240], f32, kind="ExternalOutput").ap()
        dbg_csb = nc.dram_tensor('dbg_csb', [128, 240], f32, kind="ExternalOutput").ap()
        dbg_crb = nc.dram_tensor('dbg_crb', [128, 240], f32, kind="ExternalOutput").ap()

    with tile.TileContext(nc) as tc:
        persist_cm = tc.tile_pool(name="persist", bufs=1)
        persist = persist_cm.__enter__()
        ps_cm = tc.tile_pool(name="ps", bufs=8, space="PSUM")
        ps = ps_cm.__enter__()

        def load(pool, name, shape, dt_=f32):
            t_ = pool.tile(list(shape), dt_, tag=name)
            nc.sync.dma_start(t_[:], dram[name][:])
            return t_

        # ---- persistent tensors ----
        # 32 pad cols so 60-strided win32 views in the final stage stay
        # in-bounds for the last pair
        hA = persist.tile([128, NL + 32], f32, tag="hA")
        nc.sync.dma_start(hA[:, 0:NL], dram['ht0'][:])
        nc.vector.memset(hA[:, NL:NL + 32], 0.0)
        # f32r shadow of h for matmul operands (f32r consumers require
        # producers that round; engine dtype-converting copies do).
        # One tile per 480-node group so next-layer stage A only waits on
        # its own quarter, keeping the PE fed across layer boundaries.
        hr_g = []
        for g in range(4):
            hq_t = persist.tile([128, 480], f32r, tag=f"hr{g}")
            nc.scalar.activation(hq_t[:], hA[:, 480*g:480*g+480], AF.Copy)
            hr_g.append(hq_t)

        def hr_ap(c0, c1):
            """view of h shadow cols [c0:c1) — must lie in one group"""
            g = c0 // 480
            assert c1 <= 480 * (g + 1)
            return hr_g[g][:, c0 - 480*g:c1 - 480*g]
        w1ab_s = load(persist, 'w1ab', (128, 512), f32r)
        m1_s = load(persist, 'm1', (128, 512), f32r)
        a2_s = load(persist, 'a2', (128, H), f32r)
        wu2_s = load(persist, 'wu2', (128, H), f32r)
        b2a1_s = load(persist, 'b2a1', (1, H), f32r)
        ub2_s = load(persist, 'ub2', (1, 128), f32r)
        updb1_s = load(persist, 'updb1', (128, 2))
        indeg_s = load(persist, 'indeg', (1, NL), f32r)
        ft1_s = load(persist, 'ft1', (128, T)); ft2_s = load(persist, 'ft2', (T, T))
        ft1b_s = load(persist, 'ft1b', (T, 1)); ft2b_s = load(persist, 'ft2b', (T, 1))
        onesbd_s = load(persist, 'onesbd', (128, 128))
        onesq_s = load(persist, 'onesq', (128, 4))
        ones_f = persist.tile([1, 512], f32, tag="ones_f")
        nc.vector.memset(ones_f[:], 1.0)
        ones_r = persist.tile([1, 512], f32r, tag="ones_r")
        nc.scalar.activation(ones_r[:], ones_f[:], AF.Copy)
        ident = persist.tile([128, 128], f32, tag="ident")
        make_identity(nc, ident[:])

        # ---- propagation-scoped pools ----
        mask_cm = tc.tile_pool(name="maskp", bufs=1)
        maskp = mask_cm.__enter__()
        uv_cm = tc.tile_pool(name="uvp", bufs=1)
        uvp = uv_cm.__enter__()
        agg_cm = tc.tile_pool(name="aggp", bufs=2)
        aggpool = agg_cm.__enter__()
        rg_cm = tc.tile_pool(name="rgp", bufs=2)
        rgpool = rg_cm.__enter__()
        relu_cm = tc.tile_pool(name="relu1", bufs=36)
        relu_pool = relu_cm.__enter__()

        # chunked mask DMA so layer-0 gathers can start early
        gu_a = maskp.tile([128, 24 * 128], f32r, tag="gu_a")
        gu_b = maskp.tile([128, 24 * 128], f32r, tag="gu_b")
        gv_a = maskp.tile([128, 24 * 128], f32r, tag="gv_a")
        gv_b = maskp.tile([128, 24 * 128], f32r, tag="gv_b")
        nc.sync.dma_start(gu_a[:], dram['gu'][:, 0:3072])
        nc.sync.dma_start(gv_a[:], dram['gv'][:, 0:3072])
        nc.sync.dma_start(gu_b[:], dram['gu'][:, 3072:6144])
        nc.sync.dma_start(gv_b[:], dram['gv'][:, 3072:6144])
        gs_s = maskp.tile([128, GS_COLS], f32r, tag="gs")
        nc.sync.dma_start(gs_s[:], dram['gs'][:])

        def gu_ap(idx):
            return (gu_a if idx < 24 else gu_b)[:, (idx % 24)*128:(idx % 24)*128+128]

        def gv_ap(idx):
            return (gv_a if idx < 24 else gv_b)[:, (idx % 24)*128:(idx % 24)*128+128]

        UV_s = uvp.tile([128, 16 * 512], f32r, tag="UV")
        # rows 120..127: row 120 = c1/2 constants, 121..127 zeros (stage A
        # rewrites rows 0:120 every layer before the gathers read them)
        nc.sync.dma_start(UV_s[120:128, :], dram['c1pad'][:])

        # per-tile gather incidence lists: (uv_tile_k, gu_col_idx)
        gath = {}
        for t in range(NFT):
            gath[t] = [(t // 2, t)]
        for rt in range(NRT):
            gath[NFT + rt] = [(8*rt + kk, NFT + 8*rt + kk) for kk in range(8)]

        for layer in range(N_PROP):
            # --- stage A: UV[k] = h_k @ [W1a|W1b] (rows 0:120) ---
            for k in range(16):
                pu = ps.tile([128, 512], f32, tag="ps")
                nc.tensor.matmul(pu[0:120, 0:512],
                                 lhsT=hr_ap(120*k, 120*k+120),
                                 rhs=w1ab_s[:], start=True, stop=True)
                if k % 2 == 0:
                    nc.scalar.activation(UV_s[0:120, 512*k:512*(k+1)],
                                         pu[0:120, 0:512], AF.Copy)
                else:
                    nc.vector.tensor_copy(UV_s[0:120, 512*k:512*(k+1)],
                                          pu[0:120, 0:512])

            # --- gathers + relu (remainder tiles first: groups need them) ---
            relu_t = {}
            order = [NFT, NFT + 1] + list(range(NFT))
            for t in order:
                inc = gath[t]
                pp = ps.tile([128, 512], f32, tag="ps")
                for j, (k, idx) in enumerate(inc):
                    nc.tensor.matmul(pp[:, 0:256], lhsT=gu_ap(idx),
                                     rhs=UV_s[:, 512*k:512*k+256],
                                     start=(j == 0), stop=False)
                    nc.tensor.matmul(pp[:, 0:256], lhsT=gv_ap(idx),
                                     rhs=UV_s[:, 512*k+256:512*k+512],
                                     start=False, stop=(j == len(inc) - 1))
                rt_ = relu_pool.tile([128, 256], f32r, tag="r1")
                nc.vector.tensor_relu(rt_[:], pp[:, 0:256])
                relu_t[t] = rt_

            # --- per 480-node group: scatter + update ---
            for g in range(4):
                agg_h0 = ps.tile([128, 512], f32, tag="ps")
                agg_h1 = ps.tile([128, 512], f32, tag="ps")
                aggp = [agg_h0, agg_h1]
                rt_idx = NFT + (0 if g < 2 else 1)
                rcol = NFT*120 + (2*(rt_idx - NFT) + (g % 2)) * 480
                for hh in range(2):
                    nc.tensor.matmul(aggp[hh][:, 0:480],
                                     lhsT=relu_t[rt_idx][:, 128*hh:128*hh+128],
                                     rhs=gs_s[:, rcol:rcol+480],
                                     start=True, stop=False)
                for bi in range(4):
                    b = 4*g + bi
                    for i in range(2):
                        t = 2*b + i
                        last = (bi == 3 and i == 1)
                        for hh in range(2):
                            nc.tensor.matmul(
                                aggp[hh][:, 120*bi:120*bi+120],
                                lhsT=relu_t[t][:, 128*hh:128*hh+128],
                                rhs=gs_s[:, t*120:t*120+120],
                                start=False, stop=last, skip_group_check=True)
                agg_s = aggpool.tile([128, 960], f32r, tag="agg")
                nc.scalar.activation(agg_s[:, 0:480], aggp[0][:, 0:480], AF.Copy)
                nc.vector.tensor_copy(agg_s[:, 480:960], aggp[1][:, 0:480])
                if DBG and layer == 0 and g == 0:
                    nc.sync.dma_start(dbg_agg[:], agg_s[:])

                ns = slice(480*g, 480*g+480)
                rg_s = rgpool.tile([128, 960], f32r, tag="rg")
                for hh in range(2):
                    pq = ps.tile([128, 512], f32, tag="ps")
                    nc.tensor.matmul(pq[:, 0:480], lhsT=m1_s[:, 128*hh:128*hh+128],
                                     rhs=agg_s[:, 0:480], start=True, stop=False)
                    nc.tensor.matmul(pq[:, 0:480],
                                     lhsT=m1_s[:, 256+128*hh:256+128*hh+128],
                                     rhs=agg_s[:, 480:960], start=False, stop=False)
                    nc.tensor.matmul(pq[:, 0:480], lhsT=a2_s[:, 128*hh:128*hh+128],
                                     rhs=hr_g[g][:],
                                     start=False, stop=False)
                    nc.tensor.matmul(pq[:, 0:480], lhsT=b2a1_s[0:1, 128*hh:128*hh+128],
                                     rhs=indeg_s[0:1, ns], start=False, stop=True)
                    nc.scalar.activation(rg_s[:, 480*hh:480*hh+480], pq[:, 0:480],
                                         AF.Relu, bias=updb1_s[:, hh:hh+1])
                pd = ps.tile([128, 512], f32, tag="ps")
                nc.tensor.matmul(pd[:, 0:480], lhsT=wu2_s[:, 0:128],
                                 rhs=rg_s[:, 0:480], start=True, stop=False)
                nc.tensor.matmul(pd[:, 0:480], lhsT=wu2_s[:, 128:256],
                                 rhs=rg_s[:, 480:960], start=False, stop=False)
                nc.tensor.matmul(pd[:, 0:480], lhsT=ub2_s[:],
                                 rhs=ones_r[0:1, 0:480], start=False, stop=True)
                nc.vector.tensor_add(hA[:, ns], hA[:, ns], pd[:, 0:480])
                if layer < N_PROP - 1:
                    nc.scalar.activation(hr_g[g][:], hA[:, ns], AF.Copy)

        if DBG:
            nc.sync.dma_start(dbg_h[:], hA[:, 0:NL])
            nc.sync.dma_start(dbg_uv[:], UV_s[:])
        # close propagation pools
        relu_cm.__exit__(None, None, None)
        rg_cm.__exit__(None, None, None)
        agg_cm.__exit__(None, None, None)
        uv_cm.__exit__(None, None, None)
        mask_cm.__exit__(None, None, None)

        fin_cm = tc.tile_pool(name="fin", bufs=1)
        fin = fin_cm.__enter__()
        work_cm = tc.tile_pool(name="work", bufs=4)
        work = work_cm.__enter__()

        # ---- final stage (fp32) ----
        # transforms: s1 = relu(ft1^T h + b1); tT = ft2^T s1 + b2
        s1_s = fin.tile([T, NL], f32, tag="s1")
        tT_s = fin.tile([T, NL], f32, tag="tT")
        for j in range(4):
            cs = slice(480*j, 480*(j+1))
            p1 = ps.tile([128, 512], f32, tag="ps")
            nc.tensor.matmul(p1[0:T, 0:480], lhsT=ft1_s[:], rhs=hA[:, cs],
                             start=True, stop=True)
            nc.scalar.activation(s1_s[:, cs], p1[0:T, 0:480], AF.Relu, bias=ft1b_s[:])
            p2 = ps.tile([128, 512], f32, tag="ps")
            nc.tensor.matmul(p2[0:T, 0:480], lhsT=ft2_s[:], rhs=s1_s[:, cs],
                             start=True, stop=True)
            nc.scalar.activation(tT_s[:, cs], p2[0:T, 0:480], AF.Identity,
                                 bias=ft2b_s[:])

        # masked query transform: mtq [T, BP*NC], zero at q>=NQ
        mtq_s = fin.tile([T, BP * NC], f32, tag="mtq")
        nc.vector.memset(mtq_s[:], 0.0)
        nc.vector.tensor_copy(
            mtq_s[:].rearrange("p (b n) -> p b n", n=NC)[:, :, 0:NQ],
            tT_s[:].rearrange("p (b n) -> p b n", n=NPG)[:, :, 0:NQ])

        # log-alpha: pair p=(j=p%4 row-block, g=p//4 col-group) -> [128, 240]
        pla = ps.tile([128, 512], f32, tag="ps")
        for p in range(BP):
            j, g = p % 4, p // 4
            nc.tensor.matmul(pla[32*j:32*j+30, 30*g:30*g+30],
                             lhsT=mtq_s[0:T, 30*p:30*p+30],
                             rhs=tT_s[0:T, NPG*p+NC:NPG*p+2*NC],
                             start=True, stop=True, tile_position=(0, 32*j))
        # row-max subtract (in psum), then exp(10*x) into alpha
        al_s = fin.tile([128, 240], f32, tag="al")
        nc.vector.memset(al_s[:], 1.0)
        mx_s = work.tile([128, 8], f32, tag="mx")
        pla3 = pla[:, 0:240].rearrange("p (a b) -> p a b", b=NC)
        nc.vector.tensor_reduce(mx_s[:], pla3, axis=AX.X, op=ALU.max)
        nc.vector.tensor_tensor(pla3, pla3,
                                mx_s[:, :, None].broadcast_to([128, 8, NC]),
                                op=ALU.subtract)
        for j in range(4):
            nc.scalar.activation(al_s[32*j:32*j+30, :], pla[32*j:32*j+30, 0:240],
                                 AF.Exp, scale=1.0 / SK_TEMP)

        if DBG:
            nc.sync.dma_start(dbg_al0[:], al_s[:])
        # linear-domain sinkhorn
        al3 = al_s[:].rearrange("p (a b) -> p a b", b=NC)
        rs_s = work.tile([128, 8], f32, tag="rs")
        rr_s = work.tile([128, 8], f32, tag="rr")
        crb_s = fin.tile([128, 240], f32, tag="crb")
        csb_s = fin.tile([128, 240], f32, tag="csb")
        for it in range(SK_ITERS):
            nc.vector.tensor_reduce(rs_s[:], al3, axis=AX.X, op=ALU.add)
            nc.vector.reciprocal(rr_s[:], rs_s[:])
            nc.vector.tensor_tensor(al3, al3,
                                    rr_s[:, :, None].broadcast_to([128, 8, NC]),
                                    op=ALU.mult)
            pcb = ps.tile([128, 512], f32, tag="ps")
            nc.tensor.matmul(pcb[:, 0:240], lhsT=onesbd_s[:], rhs=al_s[:],
                             start=True, stop=True)
            nc.vector.tensor_copy(csb_s[:], pcb[:, 0:240])
            nc.vector.reciprocal_approx_fast(out=crb_s[:], in_=csb_s[:])
            if DBG and it == 0:
                nc.sync.dma_start(dbg_rs[:], rs_s[:])
                nc.sync.dma_start(dbg_rr[:], rr_s[:])
                nc.sync.dma_start(dbg_alr[:], al_s[:])
                nc.sync.dma_start(dbg_csb[:], csb_s[:])
                nc.sync.dma_start(dbg_crb[:], crb_s[:])
            nc.vector.tensor_tensor(al_s[:], al_s[:], crb_s[:], op=ALU.mult)

        if DBG:
            nc.sync.dma_start(dbg_al[:], al_s[:])
        # transport-plan transposes: per col-group g, [128,30] -> [30,128]
        # (c at base 0, q of pair (j,g) on free cols 32j..32j+29)
        tpT_s = fin.tile([30, 8 * 128], f32, tag="tpT")
        for g in range(8):
            ptp = ps.tile([128, 512], f32, tag="ps")
            nc.tensor.transpose(ptp[0:30, 0:128], al_s[:, 30*g:30*g+30], ident[:])
            nc.vector.tensor_copy(tpT_s[:, 128*g:128*(g+1)], ptp[0:30, 0:128])

        # c embeddings per pair, c-major [30, 128], straight from hA
        cnm_s = fin.tile([30, BP * D], f32, tag="cnm")
        for p in range(BP):
            pc_ = ps.tile([128, 512], f32, tag="ps")
            nc.tensor.transpose(pc_[0:30, 0:128], hA[:, NPG*p+NC:NPG*p+2*NC],
                                ident[:])
            if p % 2 == 0:
                nc.scalar.activation(cnm_s[:, D*p:D*(p+1)], pc_[0:30, 0:128],
                                     AF.Copy)
            else:
                nc.vector.tensor_copy(cnm_s[:, D*p:D*(p+1)], pc_[0:30, 0:128])

        # q embeddings node-major at 32-stride (4 pairs per 128-col slab)
        qnm_s = fin.tile([128, 8 * D], f32, tag="qnm")

        def win32(off):
            w = hA[:, off:off + 240]
            return w.rearrange("p (b n) -> p b n", n=NPG)[:, :, 0:32]

        for b4 in range(8):
            stg_q = work.tile([128, 128], f32, tag="stg")
            nc.vector.tensor_copy(
                stg_q[:].rearrange("p (b n) -> p b n", n=32), win32(240*b4))
            pq_ = ps.tile([128, 512], f32, tag="ps")
            nc.tensor.transpose(pq_[0:128, 0:128], stg_q[:], ident[:])
            nc.scalar.activation(qnm_s[:, D*b4:D*(b4+1)], pq_[0:128, 0:128], AF.Copy)

        # moved = tp @ c_emb (4 pairs batched per group psum), then scores
        sd_s = fin.tile([128, 8], f32, tag="sd")
        for g in range(8):
            pm = ps.tile([128, 512], f32, tag="ps")
            nc.vector.memset(pm[:, 0:128], 0.0)
            for j in range(4):
                p = 4*g + j
                nc.tensor.matmul(pm[32*j:32*j+30, 0:128],
                                 lhsT=tpT_s[0:30, 128*g+32*j:128*g+32*j+30],
                                 rhs=cnm_s[0:30, D*p:D*(p+1)],
                                 start=True, stop=True, tile_position=(0, 32*j))
            dif = work.tile([128, 128], f32, tag="dif")
            nc.vector.tensor_sub(dif[:], qnm_s[:, D*g:D*(g+1)], pm[:, 0:128])
            nc.scalar.activation(dif[:], dif[:], AF.Relu)
            nc.vector.tensor_reduce(sd_s[:, g:g+1], dif[:], axis=AX.X, op=ALU.add)
        psc = ps.tile([128, 512], f32, tag="ps")
        nc.tensor.matmul(psc[0:4, 0:8], lhsT=onesq_s[:], rhs=sd_s[:],
                         start=True, stop=True)
        score_row = work.tile([4, 8], f32, tag="srow")
        nc.scalar.activation(score_row[:], psc[0:4, 0:8], AF.Copy, scale=-1.0)
        nc.sync.dma_start(scores_out[:], score_row[:])

        work_cm.__exit__(None, None, None)
        fin_cm.__exit__(None, None, None)
        ps_cm.__exit__(None, None, None)
        persist_cm.__exit__(None, None, None)

    nc.compile()
    return nc


def _get_program():
    if 'nc' not in _CACHE:
        _CACHE['nc'] = _build()
    return _CACHE['nc']


def kernel(**inputs) -> np.ndarray:
    from concourse.bass_utils import run_bass_kernel_spmd
    in_maps = _host_prep(inputs)
    nc = _get_program()
    res = run_bass_kernel_spmd(nc, in_maps, core_ids=list(range(NCORES)))
    out = np.zeros(B, np.float32)
    for c in range(NCORES):
        r = np.asarray(res.results[c]['scores'])   # [4, 8]
        for p in range(BP):
            out[c*BP + p] = r[p % 4, p // 4]
    return out.astype(np.float32)


# revision 37
# speedup vs baseline: 1.1188x; 1.1188x over previous
"""Trainium2 Bass kernel for nn_AddingToQ (GNN message passing + sinkhorn).

Self-contained: takes FULL unsharded inputs, shards 256 graph pairs across
8 NeuronCores (32 pairs / 1920 nodes / 4320 real edges per core), runs an
all-SBUF matmul-formulated GNN, gathers per-core scores to the full [256]
output.

v2 restructuring (validated in numpy against the jax reference):
  * all-f32r propagation (fp32 bits, 1-pass PE rate at free>=256) vs the
    old fp32 node MLPs (4 cycles/row).
  * node tiles = 2 pairs compacted to 120 rows; per-block edge tiles (2 full
    128-tiles + packed remainder) -> every full edge tile gathers from
    exactly one node tile (48 incidences/direction vs 62).
  * msg-MLP edge constant c1 folded into row 120 of the U/V tiles: the
    gather one-hots carry an extra ones-row, so c1 costs zero instructions.
  * sinkhorn runs in the linear domain (row-max exp once, then 10 rounds of
    row/col divide) -> no exp/ln alternation, 2 act-table loads total.
  * final stage stays true-fp32 (precision: the 1/temp=10x logit scale
    amplifies any rounding into the transport plan).
"""
import numpy as np

# problem constants
B, NQ, NC = 256, 15, 30
NPG = 2 * NC
N = B * NPG
EPP = 135                 # real (mask=1) edges per pair
E_REAL = B * EPP
D, H, T = 128, 256, 64
N_PROP, SK_ITERS, SK_TEMP = 5, 10, 0.1
NCORES = 8
BP = B // NCORES          # 32 pairs per core
NL = BP * NPG             # 1920 nodes per core
EL = BP * EPP             # 4320 edges per core
NBLK = BP // 2            # 16 blocks (2 pairs = 120 nodes, 270 edges)
NFT = 32                  # full edge tiles (2 per block)
NRT = 2                   # remainder tiles (8 blocks x 14 edges = 112)
ET = NFT + NRT
NGU = NFT + 16            # gather incidences per direction
GS_COLS = NFT * 120 + 4 * 480   # scatter one-hot columns

_CACHE = {}


def _host_prep(inputs):
    f32 = np.float32
    msg_w1 = np.asarray(inputs['msg_w1'], f32)
    W1a, W1b, W1c = msg_w1[0:128], msg_w1[128:256], msg_w1[256:384]
    upd_w1 = np.asarray(inputs['upd_w1'], f32)
    A1, A2 = upd_w1[0:128], upd_w1[128:256]
    msg_w2 = np.asarray(inputs['msg_w2'], f32)
    M1 = (msg_w2 @ A1).astype(f32)
    b2A1 = (np.asarray(inputs['msg_b2'], f32) @ A1).astype(f32)
    upd_b1 = np.asarray(inputs['upd_b1'], f32)
    upd_w2 = np.asarray(inputs['upd_w2'], f32)
    upd_b2 = np.asarray(inputs['upd_b2'], f32)

    nf = np.asarray(inputs['node_features'], f32)
    h0 = nf * np.asarray(inputs['enc_node_w'], f32)[0][None, :] \
        + np.asarray(inputs['enc_node_b'], f32)[None, :]
    ef = np.asarray(inputs['edge_features'], f32)
    e_enc = ef * np.asarray(inputs['enc_edge_w'], f32)[0][None, :] \
        + np.asarray(inputs['enc_edge_b'], f32)[None, :]
    C_all = (e_enc @ W1c + np.asarray(inputs['msg_b1'], f32)[None, :]).astype(f32)
    assert bool(np.all(C_all[:E_REAL] == C_all[0])), "edge encodings not uniform"
    c1h = 0.5 * C_all[0]

    from_idx = np.asarray(inputs['from_idx']).astype(np.int64)
    to_idx = np.asarray(inputs['to_idx']).astype(np.int64)
    mask = np.asarray(inputs['mask_from_idx'], f32)
    assert np.all(mask[:E_REAL] == 1.0) and np.all(mask[E_REAL:] == 0.0)
    pair_of_edge = np.arange(E_REAL) // EPP
    assert np.all(from_idx[:E_REAL] // NPG == pair_of_edge)
    assert np.all(to_idx[:E_REAL] // NPG == pair_of_edge)

    # weights in exact SBUF layouts (same for all cores)
    w1ab = np.concatenate([W1a, W1b], axis=1)                     # [128, 512]
    m1 = np.concatenate([M1[0:128], M1[128:256]], axis=1)         # [128, 512]
    wu2 = np.concatenate([upd_w2[0:128], upd_w2[128:256]], axis=1)  # [128,256]
    updb1 = np.stack([upd_b1[0:128], upd_b1[128:256]], axis=1)    # [128, 2]
    c1pad = np.zeros((8, 16 * 512), f32)    # UV rows 120..127 (row 0 = c1/2)
    for k in range(16):
        c1pad[0, 512*k:512*k+256] = c1h
        c1pad[0, 512*k+256:512*k+512] = c1h
    # sinkhorn column-sum-broadcast ones (with junk-col fix) and score ones
    onesbd = np.zeros((128, 128), f32)
    onesq = np.zeros((128, 4), f32)
    for j in range(4):
        # junk cols (s>=30) get the same pattern: block colsums are positive,
        # so junk rows stay finite across iterations
        for s in range(32):
            onesbd[32*j:32*j+30, 32*j+s] = 1.0
        onesq[32*j:32*j+30, j] = 1.0

    common = {
        'w1ab': np.ascontiguousarray(w1ab), 'm1': np.ascontiguousarray(m1),
        'a2': np.ascontiguousarray(A2), 'wu2': np.ascontiguousarray(wu2),
        'b2a1': np.ascontiguousarray(b2A1[None, :]),
        'ub2': np.ascontiguousarray(upd_b2[None, :]),
        'updb1': np.ascontiguousarray(updb1),
        'c1pad': c1pad,
        'ft1': np.ascontiguousarray(np.asarray(inputs['ft1_w'], f32)),
        'ft2': np.ascontiguousarray(np.asarray(inputs['ft2_w'], f32)),
        'ft1b': np.ascontiguousarray(np.asarray(inputs['ft1_b'], f32)[:, None]),
        'ft2b': np.ascontiguousarray(np.asarray(inputs['ft2_b'], f32)[:, None]),
        'onesbd': onesbd, 'onesq': onesq,
    }

    in_maps = []
    for c in range(NCORES):
        n0, e0 = c * NL, c * EL
        fl = from_idx[e0:e0 + EL] - n0
        tl = to_idx[e0:e0 + EL] - n0
        assert fl.min() >= 0 and fl.max() < NL and tl.min() >= 0 and tl.max() < NL

        gu = np.zeros((128, NGU * 128), f32)
        gv = np.zeros((128, NGU * 128), f32)
        gs = np.zeros((128, GS_COLS), f32)
        for t in range(NFT):
            b, i = t // 2, t % 2
            es = slice(270*b + 128*i, 270*b + 128*i + 128)
            flb, tlb = fl[es] - 120*b, tl[es] - 120*b
            cols = np.arange(128)
            gu[flb, t*128 + cols] = 1.0
            gv[tlb, t*128 + cols] = 1.0
            gu[120, t*128:(t+1)*128] = 1.0
            gv[120, t*128:(t+1)*128] = 1.0
            gs[cols, t*120 + tlb] = 1.0
        for rt in range(NRT):
            for kk in range(8):
                bb = 8*rt + kk
                idx = NFT + 8*rt + kk
                js = 14*kk + np.arange(14)
                es = 270*bb + 256 + np.arange(14)
                flb, tlb = fl[es] - 120*bb, tl[es] - 120*bb
                gu[flb, idx*128 + js] = 1.0
                gv[tlb, idx*128 + js] = 1.0
                gu[120, idx*128 + js] = 1.0
                gv[120, idx*128 + js] = 1.0
                gg = bb // 4             # target group
                gcol = NFT*120 + (2*rt + (gg % 2)) * 480
                gs[js, gcol + 120*(bb % 4) + tlb] = 1.0

        indeg = np.zeros(NL, f32)
        np.add.at(indeg, tl, 1.0)

        m = {
            'ht0': np.ascontiguousarray(h0[n0:n0 + NL].T),        # [128,1920]
            'gu': gu, 'gv': gv, 'gs': gs,
            'indeg': np.ascontiguousarray(indeg[None, :]),
        }
        m.update(common)
        in_maps.append(m)
    return in_maps


def _build():
    """Build + schedule the Bass/Tile program (identical for all cores)."""
    import concourse.bass as bass
    import concourse.tile as tile
    from concourse import bacc, mybir
    from concourse.masks import make_identity

    f32 = mybir.dt.float32
    f32r = mybir.dt.float32r
    AF = mybir.ActivationFunctionType
    ALU = mybir.AluOpType
    AX = mybir.AxisListType

    nc = bacc.Bacc("TRN2", target_bir_lowering=False, debug=False)

    dram = {}
    def din(name, shape, dt_=f32):
        dram[name] = nc.dram_tensor(name, list(shape), dt_,
                                    kind="ExternalInput").ap()
    din('ht0', (128, NL))
    din('gu', (128, NGU * 128), f32r)
    din('gv', (128, NGU * 128), f32r)
    din('gs', (128, GS_COLS), f32r)
    din('indeg', (1, NL), f32r)
    din('w1ab', (128, 512), f32r); din('m1', (128, 512), f32r)
    din('a2', (128, H), f32r); din('wu2', (128, H), f32r)
    din('b2a1', (1, H), f32r); din('ub2', (1, 128), f32r)
    din('updb1', (128, 2))
    din('c1pad', (8, 16 * 512), f32r)
    din('ft1', (128, T)); din('ft2', (T, T))
    din('ft1b', (T, 1)); din('ft2b', (T, 1))
    din('onesbd', (128, 128)); din('onesq', (128, 4))
    scores_out = nc.dram_tensor('scores', [4, 8], f32, kind="ExternalOutput").ap()
    import os
    DBG = bool(os.environ.get('KERNEL_DEBUG'))
    if DBG:
        dbg_h = nc.dram_tensor('dbg_h', [128, NL], f32, kind="ExternalOutput").ap()
        dbg_al0 = nc.dram_tensor('dbg_al0', [128, 240], f32, kind="ExternalOutput").ap()
        dbg_al = nc.dram_tensor('dbg_al', [128, 240], f32, kind="ExternalOutput").ap()
        dbg_uv = nc.dram_tensor('dbg_uv', [128, 8192], mybir.dt.float32r, kind="ExternalOutput").ap()
        dbg_agg = nc.dram_tensor('dbg_agg', [128, 960], mybir.dt.float32r, kind="ExternalOutput").ap()
        dbg_rs = nc.dram_tensor('dbg_rs', [128, 8], f32, kind="ExternalOutput").ap()
        dbg_rr = nc.dram_tensor('dbg_rr', [128, 8], f32, kind="ExternalOutput").ap()
        dbg_alr = nc.dram_tensor('dbg_alr', [128, 240], f32, kind="ExternalOutput").ap()
        dbg_csb = nc.dram_tensor('dbg_csb', [128, 240], f32, kind="ExternalOutput").ap()
        dbg_crb = nc.dram_tensor('dbg_crb', [128, 240], f32, kind="ExternalOutput").ap()

    with tile.TileContext(nc) as tc:
        persist_cm = tc.tile_pool(name="persist", bufs=1)
        persist = persist_cm.__enter__()
        ps_cm = tc.tile_pool(name="ps", bufs=8, space="PSUM")
        ps = ps_cm.__enter__()

        def load(pool, name, shape, dt_=f32):
            t_ = pool.tile(list(shape), dt_, tag=name)
            nc.sync.dma_start(t_[:], dram[name][:])
            return t_

        # ---- persistent tensors ----
        # 32 pad cols so 60-strided win32 views in the final stage stay
        # in-bounds for the last pair
        hA = persist.tile([128, NL + 32], f32, tag="hA")
        nc.sync.dma_start(hA[:, 0:NL], dram['ht0'][:])
        nc.vector.memset(hA[:, NL:NL + 32], 0.0)
        # f32r shadow of h for matmul operands (f32r consumers require
        # producers that round; engine dtype-converting copies do).
        # One tile per 480-node group so next-layer stage A only waits on
        # its own quarter, keeping the PE fed across layer boundaries.
        hr_g = []
        for g in range(4):
            hq_t = persist.tile([128, 480], f32r, tag=f"hr{g}")
            nc.scalar.activation(hq_t[:], hA[:, 480*g:480*g+480], AF.Copy)
            hr_g.append(hq_t)

        def hr_ap(c0, c1):
            """view of h shadow cols [c0:c1) — must lie in one group"""
            g = c0 // 480
            assert c1 <= 480 * (g + 1)
            return hr_g[g][:, c0 - 480*g:c1 - 480*g]
        w1ab_s = load(persist, 'w1ab', (128, 512), f32r)
        m1_s = load(persist, 'm1', (128, 512), f32r)
        a2_s = load(persist, 'a2', (128, H), f32r)
        wu2_s = load(persist, 'wu2', (128, H), f32r)
        b2a1_s = load(persist, 'b2a1', (1, H), f32r)
        ub2_s = load(persist, 'ub2', (1, 128), f32r)
        updb1_s = load(persist, 'updb1', (128, 2))
        indeg_s = load(persist, 'indeg', (1, NL), f32r)
        ft1_s = load(persist, 'ft1', (128, T)); ft2_s = load(persist, 'ft2', (T, T))
        ft1b_s = load(persist, 'ft1b', (T, 1)); ft2b_s = load(persist, 'ft2b', (T, 1))
        onesbd_s = load(persist, 'onesbd', (128, 128))
        onesq_s = load(persist, 'onesq', (128, 4))
        ones_f = persist.tile([1, 512], f32, tag="ones_f")
        nc.vector.memset(ones_f[:], 1.0)
        ones_r = persist.tile([1, 512], f32r, tag="ones_r")
        nc.scalar.activation(ones_r[:], ones_f[:], AF.Copy)
        ident = persist.tile([128, 128], f32, tag="ident")
        make_identity(nc, ident[:])

        # ---- propagation-scoped pools ----
        mask_cm = tc.tile_pool(name="maskp", bufs=1)
        maskp = mask_cm.__enter__()
        uv_cm = tc.tile_pool(name="uvp", bufs=1)
        uvp = uv_cm.__enter__()
        agg_cm = tc.tile_pool(name="aggp", bufs=2)
        aggpool = agg_cm.__enter__()
        rg_cm = tc.tile_pool(name="rgp", bufs=2)
        rgpool = rg_cm.__enter__()
        relu_cm = tc.tile_pool(name="relu1", bufs=36)
        relu_pool = relu_cm.__enter__()

        # chunked mask DMA so layer-0 gathers can start early
        gu_a = maskp.tile([128, 24 * 128], f32r, tag="gu_a")
        gu_b = maskp.tile([128, 24 * 128], f32r, tag="gu_b")
        gv_a = maskp.tile([128, 24 * 128], f32r, tag="gv_a")
        gv_b = maskp.tile([128, 24 * 128], f32r, tag="gv_b")
        nc.sync.dma_start(gu_a[:], dram['gu'][:, 0:3072])
        nc.sync.dma_start(gv_a[:], dram['gv'][:, 0:3072])
        nc.sync.dma_start(gu_b[:], dram['gu'][:, 3072:6144])
        nc.sync.dma_start(gv_b[:], dram['gv'][:, 3072:6144])
        gs_s = maskp.tile([128, GS_COLS], f32r, tag="gs")
        nc.sync.dma_start(gs_s[:], dram['gs'][:])

        def gu_ap(idx):
            return (gu_a if idx < 24 else gu_b)[:, (idx % 24)*128:(idx % 24)*128+128]

        def gv_ap(idx):
            return (gv_a if idx < 24 else gv_b)[:, (idx % 24)*128:(idx % 24)*128+128]

        # UV split into 4 quarter tiles (4 node tiles each) so gathers of a
        # quarter start as soon as its stage-A copies land, instead of
        # stalling the PE on all 16 (the per-layer bubble that re-throttled
        # the HAM clock). rows 120..127: row 120 = c1/2, 121..127 zeros.
        UV_q = []
        for q in range(4):
            uq_t = uvp.tile([128, 4 * 512], f32r, tag=f"UV{q}")
            nc.sync.dma_start(uq_t[120:128, :], dram['c1pad'][:, 2048*q:2048*(q+1)])
            UV_q.append(uq_t)

        def uv_ap(k, off, width):
            return UV_q[k // 4][:, 512*(k % 4) + off:512*(k % 4) + off + width]

        # per-tile gather incidence lists: (uv_tile_k, gu_col_idx)
        gath = {}
        for t in range(NFT):
            gath[t] = [(t // 2, t)]
        for rt in range(NRT):
            gath[NFT + rt] = [(8*rt + kk, NFT + 8*rt + kk) for kk in range(8)]

        for layer in range(N_PROP):
            # --- stage A: UV[k] = h_k @ [W1a|W1b] (rows 0:120) ---
            for k in range(16):
                pu = ps.tile([128, 512], f32, tag="ps")
                nc.tensor.matmul(pu[0:120, 0:512],
                                 lhsT=hr_ap(120*k, 120*k+120),
                                 rhs=w1ab_s[:], start=True, stop=True)
                dst = UV_q[k // 4][0:120, 512*(k % 4):512*(k % 4) + 512]
                if k % 2 == 0:
                    nc.scalar.activation(dst, pu[0:120, 0:512], AF.Copy)
                else:
                    nc.vector.tensor_copy(dst, pu[0:120, 0:512])

            # --- gathers + relu (remainder tiles first: groups need them) ---
            relu_t = {}
            order = list(range(16)) + [NFT] + list(range(16, NFT)) + [NFT + 1]
            for t in order:
                inc = gath[t]
                pp = ps.tile([128, 512], f32, tag="ps")
                for j, (k, idx) in enumerate(inc):
                    nc.tensor.matmul(pp[:, 0:256], lhsT=gu_ap(idx),
                                     rhs=uv_ap(k, 0, 256),
                                     start=(j == 0), stop=False)
                    nc.tensor.matmul(pp[:, 0:256], lhsT=gv_ap(idx),
                                     rhs=uv_ap(k, 256, 256),
                                     start=False, stop=(j == len(inc) - 1))
                rt_ = relu_pool.tile([128, 256], f32r, tag="r1")
                nc.vector.tensor_relu(rt_[:], pp[:, 0:256])
                relu_t[t] = rt_

            # --- per 480-node group: scatter + update ---
            for g in range(4):
                agg_h0 = ps.tile([128, 512], f32, tag="ps")
                agg_h1 = ps.tile([128, 512], f32, tag="ps")
                aggp = [agg_h0, agg_h1]
                rt_idx = NFT + (0 if g < 2 else 1)
                rcol = NFT*120 + (2*(rt_idx - NFT) + (g % 2)) * 480
                for hh in range(2):
                    nc.tensor.matmul(aggp[hh][:, 0:480],
                                     lhsT=relu_t[rt_idx][:, 128*hh:128*hh+128],
                                     rhs=gs_s[:, rcol:rcol+480],
                                     start=True, stop=False)
                for bi in range(4):
                    b = 4*g + bi
                    for i in range(2):
                        t = 2*b + i
                        last = (bi == 3 and i == 1)
                        for hh in range(2):
                            nc.tensor.matmul(
                                aggp[hh][:, 120*bi:120*bi+120],
                                lhsT=relu_t[t][:, 128*hh:128*hh+128],
                                rhs=gs_s[:, t*120:t*120+120],
                                start=False, stop=last, skip_group_check=True)
                agg_s = aggpool.tile([128, 960], f32r, tag="agg")
                nc.scalar.activation(agg_s[:, 0:480], aggp[0][:, 0:480], AF.Copy)
                nc.vector.tensor_copy(agg_s[:, 480:960], aggp[1][:, 0:480])
                if DBG and layer == 0 and g == 0:
                    nc.sync.dma_start(dbg_agg[:], agg_s[:])

                ns = slice(480*g, 480*g+480)
                rg_s = rgpool.tile([128, 960], f32r, tag="rg")
                for hh in range(2):
                    pq = ps.tile([128, 512], f32, tag="ps")
                    nc.tensor.matmul(pq[:, 0:480], lhsT=m1_s[:, 128*hh:128*hh+128],
                                     rhs=agg_s[:, 0:480], start=True, stop=False)
                    nc.tensor.matmul(pq[:, 0:480],
                                     lhsT=m1_s[:, 256+128*hh:256+128*hh+128],
                                     rhs=agg_s[:, 480:960], start=False, stop=False)
                    nc.tensor.matmul(pq[:, 0:480], lhsT=a2_s[:, 128*hh:128*hh+128],
                                     rhs=hr_g[g][:],
                                     start=False, stop=False)
                    nc.tensor.matmul(pq[:, 0:480], lhsT=b2a1_s[0:1, 128*hh:128*hh+128],
                                     rhs=indeg_s[0:1, ns], start=False, stop=True)
                    nc.scalar.activation(rg_s[:, 480*hh:480*hh+480], pq[:, 0:480],
                                         AF.Relu, bias=updb1_s[:, hh:hh+1])
                pd = ps.tile([128, 512], f32, tag="ps")
                nc.tensor.matmul(pd[:, 0:480], lhsT=wu2_s[:, 0:128],
                                 rhs=rg_s[:, 0:480], start=True, stop=False)
                nc.tensor.matmul(pd[:, 0:480], lhsT=wu2_s[:, 128:256],
                                 rhs=rg_s[:, 480:960], start=False, stop=False)
                nc.tensor.matmul(pd[:, 0:480], lhsT=ub2_s[:],
                                 rhs=ones_r[0:1, 0:480], start=False, stop=True)
                nc.vector.tensor_add(hA[:, ns], hA[:, ns], pd[:, 0:480])
                if layer < N_PROP - 1:
                    nc.scalar.activation(hr_g[g][:], hA[:, ns], AF.Copy)

        if DBG:
            nc.sync.dma_start(dbg_h[:], hA[:, 0:NL])
            for q in range(4):
                nc.sync.dma_start(dbg_uv[:, 2048*q:2048*(q+1)], UV_q[q][:])
        # close propagation pools
        relu_cm.__exit__(None, None, None)
        rg_cm.__exit__(None, None, None)
        agg_cm.__exit__(None, None, None)
        uv_cm.__exit__(None, None, None)
        mask_cm.__exit__(None, None, None)

        fin_cm = tc.tile_pool(name="fin", bufs=1)
        fin = fin_cm.__enter__()
        work_cm = tc.tile_pool(name="work", bufs=4)
        work = work_cm.__enter__()

        # ---- final stage (fp32) ----
        # transforms: s1 = relu(ft1^T h + b1); tT = ft2^T s1 + b2
        s1_s = fin.tile([T, NL], f32, tag="s1")
        tT_s = fin.tile([T, NL], f32, tag="tT")
        for j in range(4):
            cs = slice(480*j, 480*(j+1))
            p1 = ps.tile([128, 512], f32, tag="ps")
            nc.tensor.matmul(p1[0:T, 0:480], lhsT=ft1_s[:], rhs=hA[:, cs],
                             start=True, stop=True)
            nc.scalar.activation(s1_s[:, cs], p1[0:T, 0:480], AF.Relu, bias=ft1b_s[:])
            p2 = ps.tile([128, 512], f32, tag="ps")
            nc.tensor.matmul(p2[0:T, 0:480], lhsT=ft2_s[:], rhs=s1_s[:, cs],
                             start=True, stop=True)
            nc.scalar.activation(tT_s[:, cs], p2[0:T, 0:480], AF.Identity,
                                 bias=ft2b_s[:])

        # masked query transform: mtq [T, BP*NC], zero at q>=NQ
        mtq_s = fin.tile([T, BP * NC], f32, tag="mtq")
        nc.vector.memset(mtq_s[:], 0.0)
        nc.vector.tensor_copy(
            mtq_s[:].rearrange("p (b n) -> p b n", n=NC)[:, :, 0:NQ],
            tT_s[:].rearrange("p (b n) -> p b n", n=NPG)[:, :, 0:NQ])

        # log-alpha: pair p=(j=p%4 row-block, g=p//4 col-group) -> [128, 240]
        pla = ps.tile([128, 512], f32, tag="ps")
        for p in range(BP):
            j, g = p % 4, p // 4
            nc.tensor.matmul(pla[32*j:32*j+30, 30*g:30*g+30],
                             lhsT=mtq_s[0:T, 30*p:30*p+30],
                             rhs=tT_s[0:T, NPG*p+NC:NPG*p+2*NC],
                             start=True, stop=True, tile_position=(0, 32*j))
        # row-max subtract (in psum), then exp(10*x) into alpha
        al_s = fin.tile([128, 240], f32, tag="al")
        nc.vector.memset(al_s[:], 1.0)
        mx_s = work.tile([128, 8], f32, tag="mx")
        pla3 = pla[:, 0:240].rearrange("p (a b) -> p a b", b=NC)
        nc.vector.tensor_reduce(mx_s[:], pla3, axis=AX.X, op=ALU.max)
        nc.vector.tensor_tensor(pla3, pla3,
                                mx_s[:, :, None].broadcast_to([128, 8, NC]),
                                op=ALU.subtract)
        for j in range(4):
            nc.scalar.activation(al_s[32*j:32*j+30, :], pla[32*j:32*j+30, 0:240],
                                 AF.Exp, scale=1.0 / SK_TEMP)

        if DBG:
            nc.sync.dma_start(dbg_al0[:], al_s[:])
        # linear-domain sinkhorn
        al3 = al_s[:].rearrange("p (a b) -> p a b", b=NC)
        rs_s = work.tile([128, 8], f32, tag="rs")
        rr_s = work.tile([128, 8], f32, tag="rr")
        crb_s = fin.tile([128, 240], f32, tag="crb")
        csb_s = fin.tile([128, 240], f32, tag="csb")
        for it in range(SK_ITERS):
            nc.vector.tensor_reduce(rs_s[:], al3, axis=AX.X, op=ALU.add)
            nc.vector.reciprocal(rr_s[:], rs_s[:])
            nc.vector.tensor_tensor(al3, al3,
                                    rr_s[:, :, None].broadcast_to([128, 8, NC]),
                                    op=ALU.mult)
            pcb = ps.tile([128, 512], f32, tag="ps")
            nc.tensor.matmul(pcb[:, 0:240], lhsT=onesbd_s[:], rhs=al_s[:],
                             start=True, stop=True)
            nc.vector.tensor_copy(csb_s[:], pcb[:, 0:240])
            nc.vector.reciprocal_approx_fast(out=crb_s[:], in_=csb_s[:])
            if DBG and it == 0:
                nc.sync.dma_start(dbg_rs[:], rs_s[:])
                nc.sync.dma_start(dbg_rr[:], rr_s[:])
                nc.sync.dma_start(dbg_alr[:], al_s[:])
                nc.sync.dma_start(dbg_csb[:], csb_s[:])
                nc.sync.dma_start(dbg_crb[:], crb_s[:])
            nc.vector.tensor_tensor(al_s[:], al_s[:], crb_s[:], op=ALU.mult)

        if DBG:
            nc.sync.dma_start(dbg_al[:], al_s[:])
        # transport-plan transposes: per col-group g, [128,30] -> [30,128]
        # (c at base 0, q of pair (j,g) on free cols 32j..32j+29)
        tpT_s = fin.tile([30, 8 * 128], f32, tag="tpT")
        for g in range(8):
            ptp = ps.tile([128, 512], f32, tag="ps")
            nc.tensor.transpose(ptp[0:30, 0:128], al_s[:, 30*g:30*g+30], ident[:])
            nc.vector.tensor_copy(tpT_s[:, 128*g:128*(g+1)], ptp[0:30, 0:128])

        # c embeddings per pair, c-major [30, 128], straight from hA
        cnm_s = fin.tile([30, BP * D], f32, tag="cnm")
        for p in range(BP):
            pc_ = ps.tile([128, 512], f32, tag="ps")
            nc.tensor.transpose(pc_[0:30, 0:128], hA[:, NPG*p+NC:NPG*p+2*NC],
                                ident[:])
            if p % 2 == 0:
                nc.scalar.activation(cnm_s[:, D*p:D*(p+1)], pc_[0:30, 0:128],
                                     AF.Copy)
            else:
                nc.vector.tensor_copy(cnm_s[:, D*p:D*(p+1)], pc_[0:30, 0:128])

        # q embeddings node-major at 32-stride (4 pairs per 128-col slab)
        qnm_s = fin.tile([128, 8 * D], f32, tag="qnm")

        def win32(off):
            w = hA[:, off:off + 240]
            return w.rearrange("p (b n) -> p b n", n=NPG)[:, :, 0:32]

        for b4 in range(8):
            stg_q = work.tile([128, 128], f32, tag="stg")
            nc.vector.tensor_copy(
                stg_q[:].rearrange("p (b n) -> p b n", n=32), win32(240*b4))
            pq_ = ps.tile([128, 512], f32, tag="ps")
            nc.tensor.transpose(pq_[0:128, 0:128], stg_q[:], ident[:])
            nc.scalar.activation(qnm_s[:, D*b4:D*(b4+1)], pq_[0:128, 0:128], AF.Copy)

        # moved = tp @ c_emb (4 pairs batched per group psum), then scores
        sd_s = fin.tile([128, 8], f32, tag="sd")
        for g in range(8):
            pm = ps.tile([128, 512], f32, tag="ps")
            nc.vector.memset(pm[:, 0:128], 0.0)
            for j in range(4):
                p = 4*g + j
                nc.tensor.matmul(pm[32*j:32*j+30, 0:128],
                                 lhsT=tpT_s[0:30, 128*g+32*j:128*g+32*j+30],
                                 rhs=cnm_s[0:30, D*p:D*(p+1)],
                                 start=True, stop=True, tile_position=(0, 32*j))
            dif = work.tile([128, 128], f32, tag="dif")
            nc.vector.tensor_sub(dif[:], qnm_s[:, D*g:D*(g+1)], pm[:, 0:128])
            nc.scalar.activation(dif[:], dif[:], AF.Relu)
            nc.vector.tensor_reduce(sd_s[:, g:g+1], dif[:], axis=AX.X, op=ALU.add)
        psc = ps.tile([128, 512], f32, tag="ps")
        nc.tensor.matmul(psc[0:4, 0:8], lhsT=onesq_s[:], rhs=sd_s[:],
                         start=True, stop=True)
        score_row = work.tile([4, 8], f32, tag="srow")
        nc.scalar.activation(score_row[:], psc[0:4, 0:8], AF.Copy, scale=-1.0)
        nc.sync.dma_start(scores_out[:], score_row[:])

        work_cm.__exit__(None, None, None)
        fin_cm.__exit__(None, None, None)
        ps_cm.__exit__(None, None, None)
        persist_cm.__exit__(None, None, None)

    nc.compile()
    return nc


def _get_program():
    if 'nc' not in _CACHE:
        _CACHE['nc'] = _build()
    return _CACHE['nc']


def kernel(**inputs) -> np.ndarray:
    from concourse.bass_utils import run_bass_kernel_spmd
    in_maps = _host_prep(inputs)
    nc = _get_program()
    res = run_bass_kernel_spmd(nc, in_maps, core_ids=list(range(NCORES)))
    out = np.zeros(B, np.float32)
    for c in range(NCORES):
        r = np.asarray(res.results[c]['scores'])   # [4, 8]
        for p in range(BP):
            out[c*BP + p] = r[p % 4, p // 4]
    return out.astype(np.float32)
